# revision 15
# baseline (speedup 1.0000x reference)
"""Trainium2 Bass kernel for nn_DampedLayer (damped-oscillator SSM layer).

Math restructuring (validated on host vs the jax reference):
  The per-channel 2x2 time-invariant recurrence h_l = M h_{l-1} + F_l reduces
  to a 2nd-order scalar recurrence on z (Cayley-Hamilton):
      z_l = a z_{l-1} + b z_{l-2} + Bu_l,   a = tr M, b = -det M
      ys_l = c0 z_l + c1 z_{l-1}
  with Bu = x @ Bw, out = ys @ C2 + x*D  (Bw [H,2P], C2 [2P,H] fold the t-pair
  and the Cy[...,0]-Cy[...,1] sign).

Distribution: data-parallel over batch — 8 batch elements onto 8 NeuronCores,
no cross-core communication (scan runs over L per batch element).

v2 on-core pipeline per batch element (L=4096, H=1024, P=256):
  - x shipped as bf16 [L,H]; xT tiles [h,l] via 8 big DMA xbar-transposes
  - Bu matmuls (bf16, fp32 PSUM, kt-outer loop amortizes weight loads)
    -> Z tiles [128, 2t*L] fp32, channels on partitions
  - chunked scan: T=8 local scans (fused scalar_tensor_tensor, DVE),
    boundary levels (512 -> 64 -> 8 pairs): half0 on DVE, half1 on GPSIMD
  - per-super-chunk lvl1 fixup + ys (bf16) so Cy overlaps the scan tail
  - Cy matmuls (bf16) + x*D fused on the PSUM->SBUF copy -> out [L,H] fp32
"""

import numpy as np
import ml_dtypes

B, L, H, P = 8, 4096, 1024, 256
T1 = 8             # level-1 sub-chunk length
NC1 = L // T1      # 512 sub-chunks
NQ = NC1 // 8      # 64 level-2 blocks
NR = NQ // 8       # 8 level-3 blocks
NSC = 8            # super-chunks
LSC = L // NSC     # 512
KPAR = 82          # param columns per channel-half

_prog_cache = {}


def _host_params(steps_raw, G_raw, A_raw):
    """Per-channel recurrence constants in fp64."""
    steps_raw = steps_raw.astype(np.float64)
    G = np.maximum(G_raw.astype(np.float64), 0.0)
    A = np.maximum(A_raw.astype(np.float64), 0.0)
    steps = 1.0 / (1.0 + np.exp(-steps_raw))
    S = 1.0 + steps * G
    alpha = steps ** 2 * A / S
    alpha = 1.99 * np.tanh(alpha / 1.99)
    m11 = 1.0 / S
    m12 = -alpha / steps
    m21 = steps / S
    m22 = 1.0 - alpha
    a = m11 + m22
    b = -(m11 * m22 - m12 * m21)
    s1 = steps / S
    s2 = steps ** 2 / S
    c0 = s2
    c1 = m21 * s1 - m11 * s2
    return a, b, c0, c1


def _phi_tables(a, b, T):
    """phi1(j): hom solution with (z_-1,z_-2)=(1,0); phi2: (0,1). j=0..T-1."""
    n = a.shape[0]
    phi1 = np.zeros((T, n)); phi2 = np.zeros((T, n))
    p1m1, p1m2 = np.ones(n), np.zeros(n)
    p2m1, p2m2 = np.zeros(n), np.ones(n)
    for j in range(T):
        phi1[j] = a * p1m1 + b * p1m2
        phi2[j] = a * p2m1 + b * p2m2
        p1m2, p1m1 = p1m1, phi1[j]
        p2m2, p2m1 = p2m1, phi2[j]
    return phi1, phi2


def _r_powers(R, n):
    """R^(s+1) for s=0..n-1; R: (2,2,nch)."""
    out = np.zeros((n,) + R.shape)
    cur = R.copy()
    for s in range(n):
        out[s] = cur
        if s + 1 < n:
            cur = np.einsum('ijn,jkn->ikn', R, cur)
    return out


def _pack_params(a, b, c0, c1):
    """[128, 2*KPAR] fp32; partition = channel-within-half, col = hf*KPAR+idx."""
    phi1, phi2 = _phi_tables(a, b, T1)                       # (8, P)
    R = np.stack([np.stack([phi1[7], phi2[7]]),
                  np.stack([phi1[6], phi2[6]])])             # (2,2,P)
    RRs = _r_powers(R, 8)                                    # R^(s+1), s=0..7
    R8 = RRs[7]
    RR8s = _r_powers(R8, 8)                                  # R8^(s+1)
    cols = [c0, c1]
    cols += [phi1[j] for j in range(8)]
    cols += [phi2[j] for j in range(8)]
    for i in range(2):
        for k in range(2):
            cols += [RRs[s][i, k] for s in range(8)]
    for i in range(2):
        for k in range(2):
            cols += [RR8s[s][i, k] for s in range(8)]
    tab = np.stack(cols, axis=1)                             # (P, KPAR)
    assert tab.shape[1] == KPAR
    out = np.zeros((128, 2 * KPAR), np.float32)
    out[:, :KPAR] = tab[:128]
    out[:, KPAR:] = tab[128:]
    return out


# param column indices (within a half's block)
PC_C0, PC_C1 = 0, 1
PC_PHI1 = 2          # +j
PC_PHI2 = 10         # +j
PC_RR = {(0, 0): 18, (0, 1): 26, (1, 0): 34, (1, 1): 42}
PC_RR8 = {(0, 0): 50, (0, 1): 58, (1, 0): 66, (1, 1): 74}


def _build_program(gpsimd_ctl=True, debug=None):
    from contextlib import ExitStack
    import concourse.bacc as bacc
    import concourse.mybir as mybir
    import concourse.tile as tile

    f32 = mybir.dt.float32
    bf16 = mybir.dt.bfloat16
    MULT = mybir.AluOpType.mult
    ADD = mybir.AluOpType.add
    COPY = mybir.ActivationFunctionType.Copy

    nc = bacc.Bacc("TRN2", target_bir_lowering=False, debug=False,
                   enable_asserts=False, num_devices=8)

    xb_d = nc.dram_tensor("xb16", [L, H], bf16, kind="ExternalInput").ap()
    bw_d = nc.dram_tensor("bw", [H, 2 * P], bf16, kind="ExternalInput").ap()
    c2_d = nc.dram_tensor("c2", [2 * P, H], bf16, kind="ExternalInput").ap()
    drep_d = nc.dram_tensor("drep", [128, H], bf16, kind="ExternalInput").ap()
    par_d = nc.dram_tensor("par", [128, 2 * KPAR], f32, kind="ExternalInput").ap()
    out_d = nc.dram_tensor("out", [L, H], f32, kind="ExternalOutput").ap()
    if debug:
        zd0 = nc.dram_tensor("zd0", [128, 2 * L], f32, kind="ExternalOutput").ap()
        zd1 = nc.dram_tensor("zd1", [128, 2 * L], f32, kind="ExternalOutput").ap()

    with tile.TileContext(nc) as tc, ExitStack() as ctx:
        cpool = ctx.enter_context(tc.tile_pool(name="consts", bufs=1))
        zpool = ctx.enter_context(tc.tile_pool(name="zstate", bufs=1))
        xtp = ctx.enter_context(tc.tile_pool(name="xt", bufs=1))
        ysp = ctx.enter_context(tc.tile_pool(name="ys", bufs=2))
        scr = ctx.enter_context(tc.tile_pool(name="scratch", bufs=2))
        outp = ctx.enter_context(tc.tile_pool(name="outp", bufs=2))
        xinp = ctx.enter_context(tc.tile_pool(name="xin", bufs=2))
        xdp = ctx.enter_context(tc.tile_pool(name="xdp", bufs=2))
        pbu = ctx.enter_context(tc.tile_pool(name="pbu", bufs=4, space="PSUM"))
        pcy = ctx.enter_context(tc.tile_pool(name="pcy", bufs=4, space="PSUM"))

        # ---- constants ----
        bw_sb = cpool.tile([128, 8 * 512], bf16)   # [hp, (kt, m)]
        nc.sync.dma_start(bw_sb.rearrange("p (kt m) -> p kt m", kt=8),
                          bw_d.rearrange("(kt hp) m -> hp kt m", hp=128))
        c2_sb = cpool.tile([128, 4 * H], bf16)     # [kp, (g, h)]
        nc.sync.dma_start(c2_sb.rearrange("p (g h) -> p g h", g=4),
                          c2_d.rearrange("(g kp) h -> kp g h", kp=128))
        drep = cpool.tile([128, H], bf16)
        nc.sync.dma_start(drep[:], drep_d)
        par = cpool.tile([128, 2 * KPAR], f32)
        nc.sync.dma_start(par[:], par_d)

        def S(hf, idx):
            """[128,1] per-partition scalar for half hf."""
            c = hf * KPAR + idx
            return par[:, c:c + 1]

        # ---- state: Z[hf] [128, 2t*L] fp32, col = t*L + l ----
        z0 = zpool.tile([128, 2 * L], f32)
        z1 = zpool.tile([128, 2 * L], f32)
        zs = [z0, z1]
        als, bes, aqs, bqs = [], [], [], []
        for _hf in range(2):
            al = zpool.tile([128, 2 * (NC1 + 1)], f32, name=f"al{_hf}")
            be = zpool.tile([128, 2 * (NC1 + 1)], f32, name=f"be{_hf}")
            aq = zpool.tile([128, 2 * (NQ + 1)], f32, name=f"aq{_hf}")
            bq = zpool.tile([128, 2 * (NQ + 1)], f32, name=f"bq{_hf}")
            als.append(al); bes.append(be); aqs.append(aq); bqs.append(bq)

        stt_v = nc.vector.scalar_tensor_tensor
        stt_g = nc.gpsimd.scalar_tensor_tensor

        # ---- xT via DMA xbar transpose: one [L,128] -> [128,L] per kt ----
        xts = []
        for kt in range(8):
            xt = xtp.tile([128, L], bf16, name=f"xt_{kt}", tag=f"xt{kt}")
            nc.sync.dma_start_transpose(xt[:], xb_d[:, kt * 128:(kt + 1) * 128])
            xts.append(xt)

        # ================= phase 1: Bu matmuls =================
        # sch halves the sc-range so pass1 of the first half overlaps Bu of
        # the second; kt-outer amortizes weight loads over 4 sc's.
        for sch in range(2):
            scs = range(sch * 4, sch * 4 + 4)
            for g in range(4):
                t_idx, hf = divmod(g, 2)
                pbs = {}
                for sc in scs:
                    pbs[sc] = pbu.tile([128, 512], f32,
                                       name=f"ps_bu_{sc}_{g}", tag="psb")
                for kt in range(8):
                    for sc in scs:
                        nc.tensor.matmul(
                            pbs[sc][:],
                            bw_sb[:, kt * 512 + g * 128: kt * 512 + (g + 1) * 128],
                            xts[kt][:, sc * LSC:(sc + 1) * LSC],
                            start=(kt == 0), stop=(kt == 7))
                for sc in scs:
                    dst = zs[hf][:, t_idx * L + sc * LSC: t_idx * L + (sc + 1) * LSC]
                    nc.scalar.activation(dst, pbs[sc][:], COPY)

            # ---- level-1 local scans for this sch (DVE) ----
            for hf in range(2 if debug != "bu" else 0):
                zr = zs[hf].rearrange("p (t c j) -> p t c j", t=2, j=T1)
                cs, ce = sch * 256, (sch + 1) * 256
                tmp = scr.tile([128, 2 * 256], f32, name=f"p1tmp_{sch}_{hf}",
                               tag="p1tmp")
                tr = tmp.rearrange("p (t c) -> p t c", t=2)
                stt_v(zr[:, :, cs:ce, 1], zr[:, :, cs:ce, 0], S(hf, PC_PHI1 + 0),
                      zr[:, :, cs:ce, 1], MULT, ADD)
                for j in range(2, T1):
                    stt_v(tr[:], zr[:, :, cs:ce, j - 1], S(hf, PC_PHI1 + 0),
                          zr[:, :, cs:ce, j], MULT, ADD)
                    stt_v(zr[:, :, cs:ce, j], zr[:, :, cs:ce, j - 2],
                          S(hf, PC_PHI2 + 0), tr[:], MULT, ADD)

        if debug in ("bu", "pass1"):
            nc.sync.dma_start(zd0, z0[:])
            nc.sync.dma_start(zd1, z1[:])

        # ================= phase 2: boundary levels =================
        # half0 control on DVE, half1 on GPSIMD — independent chains run
        # concurrently.
        for hf in range(0 if debug in ("bu", "pass1") else 2):
            use_g = gpsimd_ctl and hf == 1
            eng = nc.gpsimd if use_g else nc.vector
            if not use_g:
                def stt(out, in0, scalar, in1, op0, op1, tmp=None):
                    stt_v(out, in0, scalar, in1, op0, op1)
            else:
                # Pool has no scalar_tensor_tensor: decompose via ts + tt
                def stt(out, in0, scalar, in1, op0, op1, tmp=None):
                    assert tmp is not None
                    nc.gpsimd.tensor_scalar(tmp, in0, scalar, None, op0=op0)
                    nc.gpsimd.tensor_tensor(out, tmp, in1, op=op1)
            gt2 = scr.tile([128, 2 * NQ], f32, name=f"gt2_{hf}", tag=f"gt2{hf}")
            gt2r = gt2.rearrange("p (t c) -> p t c", t=2)
            gt3 = scr.tile([128, 2 * NR], f32, name=f"gt3_{hf}", tag=f"gt3{hf}")
            gt3r = gt3.rearrange("p (t c) -> p t c", t=2)
            gt4 = scr.tile([128, 2], f32, name=f"gt4_{hf}", tag=f"gt4{hf}")
            gt4r = gt4.rearrange("p (t c) -> p t c", t=2)
            alr = als[hf].rearrange("p (t c) -> p t c", t=2)
            ber = bes[hf].rearrange("p (t c) -> p t c", t=2)
            aqr = aqs[hf].rearrange("p (t c) -> p t c", t=2)
            bqr = bqs[hf].rearrange("p (t c) -> p t c", t=2)
            tb = hf
            for padt in (alr, ber, aqr, bqr):
                eng.memset(padt[:, :, 0:1], 0.0)

            z7 = zs[hf].rearrange("p (t c j) -> p t c j", t=2, j=T1)[:, :, :, 7]
            z6 = zs[hf].rearrange("p (t c j) -> p t c j", t=2, j=T1)[:, :, :, 6]

            # ---- lvl2 pass1 ----
            eng.tensor_copy(alr[:, :, 1::8], z7[:, :, 0::8])
            eng.tensor_copy(ber[:, :, 1::8], z6[:, :, 0::8])
            t2 = scr.tile([128, 2 * NQ], f32, name=f"l2tmp_{hf}", tag=f"l2tmp{hf}")
            t2r = t2.rearrange("p (t c) -> p t c", t=2)
            for s in range(1, 8):
                stt(t2r[:], alr[:, :, s::8], S(tb, PC_RR[(0, 0)] + 0),
                    z7[:, :, s::8], MULT, ADD, tmp=gt2r[:])
                stt(alr[:, :, 1 + s::8], ber[:, :, s::8],
                    S(tb, PC_RR[(0, 1)] + 0), t2r[:], MULT, ADD, tmp=gt2r[:])
                stt(t2r[:], alr[:, :, s::8], S(tb, PC_RR[(1, 0)] + 0),
                    z6[:, :, s::8], MULT, ADD, tmp=gt2r[:])
                stt(ber[:, :, 1 + s::8], ber[:, :, s::8],
                    S(tb, PC_RR[(1, 1)] + 0), t2r[:], MULT, ADD, tmp=gt2r[:])

            # ---- lvl3 pass1 on q-ends ----
            eng.tensor_copy(aqr[:, :, 1::8], alr[:, :, 8::64])
            eng.tensor_copy(bqr[:, :, 1::8], ber[:, :, 8::64])
            t3 = scr.tile([128, 2 * NR], f32, name=f"l3tmp_{hf}", tag=f"l3tmp{hf}")
            t3r = t3.rearrange("p (t c) -> p t c", t=2)
            for s in range(1, 8):
                stt(t3r[:], aqr[:, :, s::8], S(tb, PC_RR8[(0, 0)] + 0),
                    alr[:, :, 8 + 8 * s::64], MULT, ADD, tmp=gt3r[:])
                stt(aqr[:, :, 1 + s::8], bqr[:, :, s::8],
                    S(tb, PC_RR8[(0, 1)] + 0), t3r[:], MULT, ADD, tmp=gt3r[:])
                stt(t3r[:], aqr[:, :, s::8], S(tb, PC_RR8[(1, 0)] + 0),
                    ber[:, :, 8 + 8 * s::64], MULT, ADD, tmp=gt3r[:])
                stt(bqr[:, :, 1 + s::8], bqr[:, :, s::8],
                    S(tb, PC_RR8[(1, 1)] + 0), t3r[:], MULT, ADD, tmp=gt3r[:])

            # ---- lvl4: sequential over 8 r-ends, R64 = RR8[7] ----
            t4 = scr.tile([128, 2], f32, name=f"l4tmp_{hf}", tag=f"l4tmp{hf}")
            t4r = t4.rearrange("p (t c) -> p t c", t=2)
            for r in range(1, 8):
                pcol, ccol = 8 * r, 8 * r + 8
                stt(t4r[:], aqr[:, :, pcol:pcol + 1],
                    S(tb, PC_RR8[(0, 0)] + 7), aqr[:, :, ccol:ccol + 1],
                    MULT, ADD, tmp=gt4r[:])
                stt(t4r[:, :, :], bqr[:, :, pcol:pcol + 1],
                    S(tb, PC_RR8[(0, 1)] + 7), t4r[:], MULT, ADD, tmp=gt4r[:])
                stt(bqr[:, :, ccol:ccol + 1], bqr[:, :, pcol:pcol + 1],
                    S(tb, PC_RR8[(1, 1)] + 7), bqr[:, :, ccol:ccol + 1],
                    MULT, ADD, tmp=gt4r[:])
                stt(bqr[:, :, ccol:ccol + 1], aqr[:, :, pcol:pcol + 1],
                    S(tb, PC_RR8[(1, 0)] + 7), bqr[:, :, ccol:ccol + 1],
                    MULT, ADD, tmp=gt4r[:])
                eng.tensor_copy(aqr[:, :, ccol:ccol + 1], t4r[:])

            # ---- lvl3 fixup ----
            for s in range(0, 7):
                stt(t3r[:], aqr[:, :, 0:57:8], S(tb, PC_RR8[(0, 0)] + s),
                    aqr[:, :, 1 + s::8], MULT, ADD, tmp=gt3r[:])
                stt(aqr[:, :, 1 + s::8], bqr[:, :, 0:57:8],
                    S(tb, PC_RR8[(0, 1)] + s), t3r[:], MULT, ADD, tmp=gt3r[:])
                stt(t3r[:], aqr[:, :, 0:57:8], S(tb, PC_RR8[(1, 0)] + s),
                    bqr[:, :, 1 + s::8], MULT, ADD, tmp=gt3r[:])
                stt(bqr[:, :, 1 + s::8], bqr[:, :, 0:57:8],
                    S(tb, PC_RR8[(1, 1)] + s), t3r[:], MULT, ADD, tmp=gt3r[:])

            # ---- lvl2 fixup ----
            for s in range(0, 7):
                stt(t2r[:], aqr[:, :, 0:NQ], S(tb, PC_RR[(0, 0)] + s),
                    alr[:, :, 1 + s::8], MULT, ADD, tmp=gt2r[:])
                stt(alr[:, :, 1 + s::8], bqr[:, :, 0:NQ],
                    S(tb, PC_RR[(0, 1)] + s), t2r[:], MULT, ADD, tmp=gt2r[:])
                stt(t2r[:], aqr[:, :, 0:NQ], S(tb, PC_RR[(1, 0)] + s),
                    ber[:, :, 1 + s::8], MULT, ADD, tmp=gt2r[:])
                stt(ber[:, :, 1 + s::8], bqr[:, :, 0:NQ],
                    S(tb, PC_RR[(1, 1)] + s), t2r[:], MULT, ADD, tmp=gt2r[:])
            eng.tensor_copy(alr[:, :, 8::8], aqr[:, :, 1:NQ + 1])
            eng.tensor_copy(ber[:, :, 8::8], bqr[:, :, 1:NQ + 1])

        # ---- lvl1 fixup per super-chunk (DVE) + ys ----
        ys_tiles = {}
        for sc in range(0 if debug in ("bu", "pass1") else NSC):
            for hf in range(2):
                alr = als[hf].rearrange("p (t c) -> p t c", t=2)
                ber = bes[hf].rearrange("p (t c) -> p t c", t=2)
                zr1 = zs[hf].rearrange("p (t c j) -> p t c j", t=2, j=T1)
                cs, ce = sc * 64, (sc + 1) * 64
                tf = scr.tile([128, 2 * 64], f32, name=f"f1tmp_{sc}_{hf}",
                              tag="f1tmp")
                tfr = tf.rearrange("p (t c) -> p t c", t=2)
                for j in range(T1):
                    stt_v(tfr[:], alr[:, :, cs:ce], S(hf, PC_PHI1 + j),
                          zr1[:, :, cs:ce, j], MULT, ADD)
                    stt_v(zr1[:, :, cs:ce, j], ber[:, :, cs:ce],
                          S(hf, PC_PHI2 + j), tfr[:], MULT, ADD)

                # ys for this (sc, hf): c1-mult on ACT, fused add on DVE
                ys = ysp.tile([128, 2 * LSC], bf16, name=f"ys_{sc}_{hf}",
                              tag=f"ys{hf}")
                for t_idx in range(2):
                    zsl = zs[hf][:, t_idx * L + sc * LSC:
                                 t_idx * L + (sc + 1) * LSC]
                    ysl = ys[:, t_idx * LSC:(t_idx + 1) * LSC]
                    tys = scr.tile([128, LSC], f32,
                                   name=f"ystmp_{sc}_{hf}_{t_idx}", tag="ystmp")
                    if sc == 0:
                        nc.vector.memset(tys[:, 0:1], 0.0)
                        nc.scalar.activation(tys[:, 1:], zsl[:, 0:LSC - 1],
                                             COPY, scale=S(hf, PC_C1))
                    else:
                        zprev = zs[hf][:, t_idx * L + sc * LSC - 1:
                                       t_idx * L + (sc + 1) * LSC - 1]
                        nc.scalar.activation(tys[:], zprev, COPY,
                                             scale=S(hf, PC_C1))
                    stt_v(ysl, zsl, S(hf, PC_C0), tys[:], MULT, ADD)
                ys_tiles[(sc, hf)] = ys

        if debug == "z":
            nc.sync.dma_start(zd0, z0[:])
            nc.sync.dma_start(zd1, z1[:])

        # ================= phase 3: Cy + x*D + out =================
        for lt in range(0 if debug in ("bu", "pass1") else L // 128):
            sc, ls = divmod(lt, 4)
            xi2 = xinp.tile([128, H], bf16, name=f"xi2_{lt}", tag="xi")
            nc.sync.dma_start(xi2[:], xb_d[lt * 128:(lt + 1) * 128, :])
            xd = xdp.tile([128, H], f32, name=f"xd_{lt}", tag="xd")
            nc.gpsimd.tensor_tensor(xd[:], xi2[:], drep[:], op=MULT)
            ob = outp.tile([128, H], f32, name=f"ob_{lt}", tag="ob")
            for nh in range(2):
                pc = pcy.tile([128, 512], f32, name=f"ps_cy_{lt}_{nh}", tag="psc")
                for g in range(4):
                    t_idx, hf = divmod(g, 2)
                    ys = ys_tiles[(sc, hf)]
                    nc.tensor.matmul(
                        pc[:],
                        ys[:, t_idx * LSC + ls * 128: t_idx * LSC + (ls + 1) * 128],
                        c2_sb[:, g * H + nh * 512: g * H + nh * 512 + 512],
                        start=(g == 0), stop=(g == 3))
                stt_v(ob[:, nh * 512:(nh + 1) * 512], pc[:], 1.0,
                      xd[:, nh * 512:(nh + 1) * 512], MULT, ADD)
            nc.sync.dma_start(out_d[lt * 128:(lt + 1) * 128, :], ob[:])

    nc.finalize()
    return nc


def _get_program():
    import os
    if "nc" not in _prog_cache:
        g = os.environ.get("K_GPSIMD_CTL", "1") == "1"
        _prog_cache["nc"] = _build_program(gpsimd_ctl=g)
    return _prog_cache["nc"]


def make_in_maps(input_sequence, steps_raw, G_raw, A_raw, B_mat, C_mat, D_vec):
    a, b, c0, c1 = _host_params(steps_raw, G_raw, A_raw)
    par = _pack_params(a, b, c0, c1)
    Bm = B_mat.astype(np.float64)   # (P, H, 2)
    Cm = C_mat.astype(np.float64)   # (H, P, 2)
    Bw = np.concatenate([Bm[:, :, 0].T, Bm[:, :, 1].T], axis=1)  # (H, 2P)
    C2 = np.concatenate([Cm[:, :, 0].T, -Cm[:, :, 1].T], axis=0)  # (2P, H)
    bw16 = np.ascontiguousarray(Bw.astype(ml_dtypes.bfloat16))
    c216 = np.ascontiguousarray(C2.astype(ml_dtypes.bfloat16))
    drep = np.broadcast_to(
        D_vec.astype(np.float32).astype(ml_dtypes.bfloat16), (128, H)).copy()
    xb16 = np.ascontiguousarray(
        np.asarray(input_sequence, np.float32).astype(ml_dtypes.bfloat16))
    in_maps = []
    for i in range(B):
        in_maps.append({
            "xb16": xb16[i],
            "bw": bw16,
            "c2": c216,
            "drep": drep,
            "par": par,
        })
    return in_maps


def kernel(input_sequence, steps_raw, G_raw, A_raw, B_mat, C_mat, D_vec):
    from concourse.bass_utils import run_bass_kernel_spmd

    nc = _get_program()
    in_maps = make_in_maps(input_sequence, steps_raw, G_raw, A_raw,
                           B_mat, C_mat, D_vec)
    res = run_bass_kernel_spmd(nc, in_maps, core_ids=list(range(B)))
    out = np.stack([res.results[i]["out"] for i in range(B)], axis=0)
    return out.astype(np.float32)


# revision 17
# speedup vs baseline: 1.4210x; 1.4210x over previous
"""Trainium2 Bass kernel for nn_DampedLayer (damped-oscillator SSM layer).

Math restructuring (validated on host vs the jax reference):
  The per-channel 2x2 time-invariant recurrence h_l = M h_{l-1} + F_l reduces
  to a 2nd-order scalar recurrence on z (Cayley-Hamilton):
      z_l = a z_{l-1} + b z_{l-2} + Bu_l,   a = tr M, b = -det M
      ys_l = c0 z_l + c1 z_{l-1}
  with Bu = x @ Bw, out = ys @ C2 + x*D  (Bw [H,2P], C2 [2P,H] fold the t-pair
  and the Cy[...,0]-Cy[...,1] sign).

Distribution: data-parallel over batch — 8 batch elements onto 8 NeuronCores,
no cross-core communication (scan runs over L per batch element).

v2 on-core pipeline per batch element (L=4096, H=1024, P=256):
  - x shipped as bf16 [L,H]; xT tiles [h,l] via 8 big DMA xbar-transposes
  - Bu matmuls (bf16, fp32 PSUM, kt-outer loop amortizes weight loads)
    -> Z tiles [128, 2t*L] fp32, channels on partitions
  - chunked scan: T=8 local scans (fused scalar_tensor_tensor, DVE),
    boundary levels (512 -> 64 -> 8 pairs): half0 on DVE, half1 on GPSIMD
  - per-super-chunk lvl1 fixup + ys (bf16) so Cy overlaps the scan tail
  - Cy matmuls (bf16) + x*D fused on the PSUM->SBUF copy -> out [L,H] fp32
"""

import numpy as np
import ml_dtypes

B, L, H, P = 8, 4096, 1024, 256
T1 = 8             # level-1 sub-chunk length
NC1 = L // T1      # 512 sub-chunks
NQ = NC1 // 8      # 64 level-2 blocks
NR = NQ // 8       # 8 level-3 blocks
NSC = 8            # super-chunks
LSC = L // NSC     # 512
KPAR = 82          # param columns per channel-half

_prog_cache = {}


def _host_params(steps_raw, G_raw, A_raw):
    """Per-channel recurrence constants in fp64."""
    steps_raw = steps_raw.astype(np.float64)
    G = np.maximum(G_raw.astype(np.float64), 0.0)
    A = np.maximum(A_raw.astype(np.float64), 0.0)
    steps = 1.0 / (1.0 + np.exp(-steps_raw))
    S = 1.0 + steps * G
    alpha = steps ** 2 * A / S
    alpha = 1.99 * np.tanh(alpha / 1.99)
    m11 = 1.0 / S
    m12 = -alpha / steps
    m21 = steps / S
    m22 = 1.0 - alpha
    a = m11 + m22
    b = -(m11 * m22 - m12 * m21)
    s1 = steps / S
    s2 = steps ** 2 / S
    c0 = s2
    c1 = m21 * s1 - m11 * s2
    return a, b, c0, c1


def _phi_tables(a, b, T):
    """phi1(j): hom solution with (z_-1,z_-2)=(1,0); phi2: (0,1). j=0..T-1."""
    n = a.shape[0]
    phi1 = np.zeros((T, n)); phi2 = np.zeros((T, n))
    p1m1, p1m2 = np.ones(n), np.zeros(n)
    p2m1, p2m2 = np.zeros(n), np.ones(n)
    for j in range(T):
        phi1[j] = a * p1m1 + b * p1m2
        phi2[j] = a * p2m1 + b * p2m2
        p1m2, p1m1 = p1m1, phi1[j]
        p2m2, p2m1 = p2m1, phi2[j]
    return phi1, phi2


def _r_powers(R, n):
    """R^(s+1) for s=0..n-1; R: (2,2,nch)."""
    out = np.zeros((n,) + R.shape)
    cur = R.copy()
    for s in range(n):
        out[s] = cur
        if s + 1 < n:
            cur = np.einsum('ijn,jkn->ikn', R, cur)
    return out


def _pack_params(a, b, c0, c1):
    """[128, 2*KPAR] fp32; partition = channel-within-half, col = hf*KPAR+idx."""
    phi1, phi2 = _phi_tables(a, b, T1)                       # (8, P)
    R = np.stack([np.stack([phi1[7], phi2[7]]),
                  np.stack([phi1[6], phi2[6]])])             # (2,2,P)
    RRs = _r_powers(R, 8)                                    # R^(s+1), s=0..7
    R8 = RRs[7]
    RR8s = _r_powers(R8, 8)                                  # R8^(s+1)
    cols = [c0, c1]
    cols += [phi1[j] for j in range(8)]
    cols += [phi2[j] for j in range(8)]
    for i in range(2):
        for k in range(2):
            cols += [RRs[s][i, k] for s in range(8)]
    for i in range(2):
        for k in range(2):
            cols += [RR8s[s][i, k] for s in range(8)]
    tab = np.stack(cols, axis=1)                             # (P, KPAR)
    assert tab.shape[1] == KPAR
    out = np.zeros((128, 2 * KPAR), np.float32)
    out[:, :KPAR] = tab[:128]
    out[:, KPAR:] = tab[128:]
    return out


# param column indices (within a half's block)
PC_C0, PC_C1 = 0, 1
PC_PHI1 = 2          # +j
PC_PHI2 = 10         # +j
PC_RR = {(0, 0): 18, (0, 1): 26, (1, 0): 34, (1, 1): 42}
PC_RR8 = {(0, 0): 50, (0, 1): 58, (1, 0): 66, (1, 1): 74}


def _build_program(gpsimd_ctl=True, debug=None):
    from contextlib import ExitStack
    import concourse.bacc as bacc
    import concourse.mybir as mybir
    import concourse.tile as tile

    f32 = mybir.dt.float32
    bf16 = mybir.dt.bfloat16
    MULT = mybir.AluOpType.mult
    ADD = mybir.AluOpType.add
    COPY = mybir.ActivationFunctionType.Copy

    nc = bacc.Bacc("TRN2", target_bir_lowering=False, debug=False,
                   enable_asserts=False, num_devices=8)

    xb_d = nc.dram_tensor("xb16", [L, H], bf16, kind="ExternalInput").ap()
    bw_d = nc.dram_tensor("bw", [H, 2 * P], bf16, kind="ExternalInput").ap()
    c2_d = nc.dram_tensor("c2", [2 * P, H], bf16, kind="ExternalInput").ap()
    drep_d = nc.dram_tensor("drep", [128, H], bf16, kind="ExternalInput").ap()
    par_d = nc.dram_tensor("par", [128, 2 * KPAR], f32, kind="ExternalInput").ap()
    out_d = nc.dram_tensor("out", [L, H], f32, kind="ExternalOutput").ap()
    if debug:
        zd0 = nc.dram_tensor("zd0", [128, 2 * L], f32, kind="ExternalOutput").ap()
        zd1 = nc.dram_tensor("zd1", [128, 2 * L], f32, kind="ExternalOutput").ap()

    with tile.TileContext(nc) as tc, ExitStack() as ctx:
        cpool = ctx.enter_context(tc.tile_pool(name="consts", bufs=1))
        zpool = ctx.enter_context(tc.tile_pool(name="zstate", bufs=1))
        xtp = ctx.enter_context(tc.tile_pool(name="xt", bufs=1))
        ysp = ctx.enter_context(tc.tile_pool(name="ys", bufs=2))
        scr = ctx.enter_context(tc.tile_pool(name="scratch", bufs=2))
        outp = ctx.enter_context(tc.tile_pool(name="outp", bufs=2))
        xinp = ctx.enter_context(tc.tile_pool(name="xin", bufs=2))
        xdp = ctx.enter_context(tc.tile_pool(name="xdp", bufs=2))
        pbu = ctx.enter_context(tc.tile_pool(name="pbu", bufs=4, space="PSUM"))
        pcy = ctx.enter_context(tc.tile_pool(name="pcy", bufs=4, space="PSUM"))

        # ---- constants ----
        bw_sb = cpool.tile([128, 8 * 512], bf16)   # [hp, (kt, m)]
        nc.sync.dma_start(bw_sb.rearrange("p (kt m) -> p kt m", kt=8),
                          bw_d.rearrange("(kt hp) m -> hp kt m", hp=128))
        c2_sb = cpool.tile([128, 4 * H], bf16)     # [kp, (g, h)]
        nc.sync.dma_start(c2_sb.rearrange("p (g h) -> p g h", g=4),
                          c2_d.rearrange("(g kp) h -> kp g h", kp=128))
        drep = cpool.tile([128, H], bf16)
        nc.sync.dma_start(drep[:], drep_d)
        par = cpool.tile([128, 2 * KPAR], f32)
        nc.sync.dma_start(par[:], par_d)

        def S(hf, idx):
            """[128,1] per-partition scalar for half hf."""
            c = hf * KPAR + idx
            return par[:, c:c + 1]

        # ---- state: Z[hf] [128, 2t*L] fp32, col = t*L + l ----
        z0 = zpool.tile([128, 2 * L], f32)
        z1 = zpool.tile([128, 2 * L], f32)
        zs = [z0, z1]
        als, bes, aqs, bqs = [], [], [], []
        for _hf in range(2):
            al = zpool.tile([128, 2 * (NC1 + 1)], f32, name=f"al{_hf}")
            be = zpool.tile([128, 2 * (NC1 + 1)], f32, name=f"be{_hf}")
            aq = zpool.tile([128, 2 * (NQ + 1)], f32, name=f"aq{_hf}")
            bq = zpool.tile([128, 2 * (NQ + 1)], f32, name=f"bq{_hf}")
            als.append(al); bes.append(be); aqs.append(aq); bqs.append(bq)

        stt_v = nc.vector.scalar_tensor_tensor
        stt_g = nc.gpsimd.scalar_tensor_tensor

        # ---- xT via DMA xbar transpose: one [L,128] -> [128,L] per kt ----
        xts = []
        for kt in range(8):
            xt = xtp.tile([128, L], bf16, name=f"xt_{kt}", tag=f"xt{kt}")
            nc.sync.dma_start_transpose(xt[:], xb_d[:, kt * 128:(kt + 1) * 128])
            xts.append(xt)

        # ================= phase 1: Bu matmuls =================
        # sch halves the sc-range so pass1 of the first half overlaps Bu of
        # the second; kt-outer amortizes weight loads over 4 sc's.
        for sch in range(2):
            scs = range(sch * 4, sch * 4 + 4)
            for g in range(4):
                t_idx, hf = divmod(g, 2)
                pbs = {}
                for sc in scs:
                    pbs[sc] = pbu.tile([128, 512], f32,
                                       name=f"ps_bu_{sc}_{g}", tag="psb")
                for kt in range(8):
                    for sc in scs:
                        nc.tensor.matmul(
                            pbs[sc][:],
                            bw_sb[:, kt * 512 + g * 128: kt * 512 + (g + 1) * 128],
                            xts[kt][:, sc * LSC:(sc + 1) * LSC],
                            start=(kt == 0), stop=(kt == 7))
                for sc in scs:
                    dst = zs[hf][:, t_idx * L + sc * LSC: t_idx * L + (sc + 1) * LSC]
                    nc.scalar.activation(dst, pbs[sc][:], COPY)

            # ---- level-1 local scans for this sch (DVE) ----
            for hf in range(2 if debug != "bu" else 0):
                zr = zs[hf].rearrange("p (t c j) -> p t c j", t=2, j=T1)
                cs, ce = sch * 256, (sch + 1) * 256
                tmp = scr.tile([128, 2 * 256], f32, name=f"p1tmp_{sch}_{hf}",
                               tag="p1tmp")
                tr = tmp.rearrange("p (t c) -> p t c", t=2)
                stt_v(zr[:, :, cs:ce, 1], zr[:, :, cs:ce, 0], S(hf, PC_PHI1 + 0),
                      zr[:, :, cs:ce, 1], MULT, ADD)
                for j in range(2, T1):
                    stt_v(tr[:], zr[:, :, cs:ce, j - 1], S(hf, PC_PHI1 + 0),
                          zr[:, :, cs:ce, j], MULT, ADD)
                    stt_v(zr[:, :, cs:ce, j], zr[:, :, cs:ce, j - 2],
                          S(hf, PC_PHI2 + 0), tr[:], MULT, ADD)

        if debug in ("bu", "pass1"):
            nc.sync.dma_start(zd0, z0[:])
            nc.sync.dma_start(zd1, z1[:])

        # ================= phase 2: boundary levels =================
        # half0 control on DVE, half1 on GPSIMD — independent chains run
        # concurrently.
        for hf in range(0 if debug in ("bu", "pass1") else 2):
            use_g = gpsimd_ctl and hf == 1
            eng = nc.gpsimd if use_g else nc.vector
            if not use_g:
                def stt(out, in0, scalar, in1, op0, op1, tmp=None):
                    stt_v(out, in0, scalar, in1, op0, op1)
            else:
                # Pool has no scalar_tensor_tensor: decompose via ts + tt
                def stt(out, in0, scalar, in1, op0, op1, tmp=None):
                    assert tmp is not None
                    nc.gpsimd.tensor_scalar(tmp, in0, scalar, None, op0=op0)
                    nc.gpsimd.tensor_tensor(out, tmp, in1, op=op1)
            gt2 = scr.tile([128, 2 * NQ], f32, name=f"gt2_{hf}", tag=f"gt2{hf}")
            gt2r = gt2.rearrange("p (t c) -> p t c", t=2)
            gt3 = scr.tile([128, 2 * NR], f32, name=f"gt3_{hf}", tag=f"gt3{hf}")
            gt3r = gt3.rearrange("p (t c) -> p t c", t=2)
            gt4 = scr.tile([128, 2], f32, name=f"gt4_{hf}", tag=f"gt4{hf}")
            gt4r = gt4.rearrange("p (t c) -> p t c", t=2)
            alr = als[hf].rearrange("p (t c) -> p t c", t=2)
            ber = bes[hf].rearrange("p (t c) -> p t c", t=2)
            aqr = aqs[hf].rearrange("p (t c) -> p t c", t=2)
            bqr = bqs[hf].rearrange("p (t c) -> p t c", t=2)
            tb = hf
            for padt in (alr, ber, aqr, bqr):
                eng.memset(padt[:, :, 0:1], 0.0)

            z7 = zs[hf].rearrange("p (t c j) -> p t c j", t=2, j=T1)[:, :, :, 7]
            z6 = zs[hf].rearrange("p (t c j) -> p t c j", t=2, j=T1)[:, :, :, 6]

            # ---- lvl2 pass1 ----
            eng.tensor_copy(alr[:, :, 1::8], z7[:, :, 0::8])
            eng.tensor_copy(ber[:, :, 1::8], z6[:, :, 0::8])
            t2 = scr.tile([128, 2 * NQ], f32, name=f"l2tmp_{hf}", tag=f"l2tmp{hf}")
            t2r = t2.rearrange("p (t c) -> p t c", t=2)
            for s in range(1, 8):
                stt(t2r[:], alr[:, :, s::8], S(tb, PC_RR[(0, 0)] + 0),
                    z7[:, :, s::8], MULT, ADD, tmp=gt2r[:])
                stt(alr[:, :, 1 + s::8], ber[:, :, s::8],
                    S(tb, PC_RR[(0, 1)] + 0), t2r[:], MULT, ADD, tmp=gt2r[:])
                stt(t2r[:], alr[:, :, s::8], S(tb, PC_RR[(1, 0)] + 0),
                    z6[:, :, s::8], MULT, ADD, tmp=gt2r[:])
                stt(ber[:, :, 1 + s::8], ber[:, :, s::8],
                    S(tb, PC_RR[(1, 1)] + 0), t2r[:], MULT, ADD, tmp=gt2r[:])

            # ---- lvl3 pass1 on q-ends ----
            eng.tensor_copy(aqr[:, :, 1::8], alr[:, :, 8::64])
            eng.tensor_copy(bqr[:, :, 1::8], ber[:, :, 8::64])
            t3 = scr.tile([128, 2 * NR], f32, name=f"l3tmp_{hf}", tag=f"l3tmp{hf}")
            t3r = t3.rearrange("p (t c) -> p t c", t=2)
            for s in range(1, 8):
                stt(t3r[:], aqr[:, :, s::8], S(tb, PC_RR8[(0, 0)] + 0),
                    alr[:, :, 8 + 8 * s::64], MULT, ADD, tmp=gt3r[:])
                stt(aqr[:, :, 1 + s::8], bqr[:, :, s::8],
                    S(tb, PC_RR8[(0, 1)] + 0), t3r[:], MULT, ADD, tmp=gt3r[:])
                stt(t3r[:], aqr[:, :, s::8], S(tb, PC_RR8[(1, 0)] + 0),
                    ber[:, :, 8 + 8 * s::64], MULT, ADD, tmp=gt3r[:])
                stt(bqr[:, :, 1 + s::8], bqr[:, :, s::8],
                    S(tb, PC_RR8[(1, 1)] + 0), t3r[:], MULT, ADD, tmp=gt3r[:])

            # ---- lvl4: sequential over 8 r-ends, R64 = RR8[7] ----
            t4 = scr.tile([128, 2], f32, name=f"l4tmp_{hf}", tag=f"l4tmp{hf}")
            t4r = t4.rearrange("p (t c) -> p t c", t=2)
            for r in range(1, 8):
                pcol, ccol = 8 * r, 8 * r + 8
                stt(t4r[:], aqr[:, :, pcol:pcol + 1],
                    S(tb, PC_RR8[(0, 0)] + 7), aqr[:, :, ccol:ccol + 1],
                    MULT, ADD, tmp=gt4r[:])
                stt(t4r[:, :, :], bqr[:, :, pcol:pcol + 1],
                    S(tb, PC_RR8[(0, 1)] + 7), t4r[:], MULT, ADD, tmp=gt4r[:])
                stt(bqr[:, :, ccol:ccol + 1], bqr[:, :, pcol:pcol + 1],
                    S(tb, PC_RR8[(1, 1)] + 7), bqr[:, :, ccol:ccol + 1],
                    MULT, ADD, tmp=gt4r[:])
                stt(bqr[:, :, ccol:ccol + 1], aqr[:, :, pcol:pcol + 1],
                    S(tb, PC_RR8[(1, 0)] + 7), bqr[:, :, ccol:ccol + 1],
                    MULT, ADD, tmp=gt4r[:])
                eng.tensor_copy(aqr[:, :, ccol:ccol + 1], t4r[:])

            # ---- lvl3 fixup ----
            for s in range(0, 7):
                stt(t3r[:], aqr[:, :, 0:57:8], S(tb, PC_RR8[(0, 0)] + s),
                    aqr[:, :, 1 + s::8], MULT, ADD, tmp=gt3r[:])
                stt(aqr[:, :, 1 + s::8], bqr[:, :, 0:57:8],
                    S(tb, PC_RR8[(0, 1)] + s), t3r[:], MULT, ADD, tmp=gt3r[:])
                stt(t3r[:], aqr[:, :, 0:57:8], S(tb, PC_RR8[(1, 0)] + s),
                    bqr[:, :, 1 + s::8], MULT, ADD, tmp=gt3r[:])
                stt(bqr[:, :, 1 + s::8], bqr[:, :, 0:57:8],
                    S(tb, PC_RR8[(1, 1)] + s), t3r[:], MULT, ADD, tmp=gt3r[:])

            # ---- lvl2 fixup ----
            for s in range(0, 7):
                stt(t2r[:], aqr[:, :, 0:NQ], S(tb, PC_RR[(0, 0)] + s),
                    alr[:, :, 1 + s::8], MULT, ADD, tmp=gt2r[:])
                stt(alr[:, :, 1 + s::8], bqr[:, :, 0:NQ],
                    S(tb, PC_RR[(0, 1)] + s), t2r[:], MULT, ADD, tmp=gt2r[:])
                stt(t2r[:], aqr[:, :, 0:NQ], S(tb, PC_RR[(1, 0)] + s),
                    ber[:, :, 1 + s::8], MULT, ADD, tmp=gt2r[:])
                stt(ber[:, :, 1 + s::8], bqr[:, :, 0:NQ],
                    S(tb, PC_RR[(1, 1)] + s), t2r[:], MULT, ADD, tmp=gt2r[:])
            eng.tensor_copy(alr[:, :, 8::8], aqr[:, :, 1:NQ + 1])
            eng.tensor_copy(ber[:, :, 8::8], bqr[:, :, 1:NQ + 1])

        # ---- lvl1 fixup per sch-half (DVE), then per-sc ys ----
        for sch in range(0 if debug in ("bu", "pass1") else 2):
            for hf in range(2):
                alr = als[hf].rearrange("p (t c) -> p t c", t=2)
                ber = bes[hf].rearrange("p (t c) -> p t c", t=2)
                zr1 = zs[hf].rearrange("p (t c j) -> p t c j", t=2, j=T1)
                cs, ce = sch * 256, (sch + 1) * 256
                tf = scr.tile([128, 2 * 256], f32, name=f"f1tmp_{sch}_{hf}",
                              tag="p1tmp")
                tfr = tf.rearrange("p (t c) -> p t c", t=2)
                for j in range(T1):
                    stt_v(tfr[:], alr[:, :, cs:ce], S(hf, PC_PHI1 + j),
                          zr1[:, :, cs:ce, j], MULT, ADD)
                    stt_v(zr1[:, :, cs:ce, j], ber[:, :, cs:ce],
                          S(hf, PC_PHI2 + j), tfr[:], MULT, ADD)

        ys_tiles = {}
        for sc in range(0 if debug in ("bu", "pass1") else NSC):
            for hf in range(2):
                # ys for this (sc, hf): c1-mult on ACT, fused add on DVE
                ys = ysp.tile([128, 2 * LSC], bf16, name=f"ys_{sc}_{hf}",
                              tag=f"ys{hf}")
                for t_idx in range(2):
                    zsl = zs[hf][:, t_idx * L + sc * LSC:
                                 t_idx * L + (sc + 1) * LSC]
                    ysl = ys[:, t_idx * LSC:(t_idx + 1) * LSC]
                    tys = scr.tile([128, LSC], f32,
                                   name=f"ystmp_{sc}_{hf}_{t_idx}", tag="ystmp")
                    if sc == 0:
                        nc.vector.memset(tys[:, 0:1], 0.0)
                        nc.scalar.activation(tys[:, 1:], zsl[:, 0:LSC - 1],
                                             COPY, scale=S(hf, PC_C1))
                    else:
                        zprev = zs[hf][:, t_idx * L + sc * LSC - 1:
                                       t_idx * L + (sc + 1) * LSC - 1]
                        nc.scalar.activation(tys[:], zprev, COPY,
                                             scale=S(hf, PC_C1))
                    stt_v(ysl, zsl, S(hf, PC_C0), tys[:], MULT, ADD)
                ys_tiles[(sc, hf)] = ys

        if debug == "z":
            nc.sync.dma_start(zd0, z0[:])
            nc.sync.dma_start(zd1, z1[:])

        # ================= phase 3: Cy + x*D + out =================
        for lt in range(0 if debug in ("bu", "pass1") else L // 128):
            sc, ls = divmod(lt, 4)
            xi2 = xinp.tile([128, H], bf16, name=f"xi2_{lt}", tag="xi")
            nc.sync.dma_start(xi2[:], xb_d[lt * 128:(lt + 1) * 128, :])
            xd = xdp.tile([128, H], f32, name=f"xd_{lt}", tag="xd")
            nc.gpsimd.tensor_tensor(xd[:], xi2[:], drep[:], op=MULT)
            ob = outp.tile([128, H], f32, name=f"ob_{lt}", tag="ob")
            pcs = [pcy.tile([128, 512], f32, name=f"ps_cy_{lt}_{nh}", tag="psc")
                   for nh in range(2)]
            for g in range(4):
                t_idx, hf = divmod(g, 2)
                ys = ys_tiles[(sc, hf)]
                for nh in range(2):
                    nc.tensor.matmul(
                        pcs[nh][:],
                        ys[:, t_idx * LSC + ls * 128: t_idx * LSC + (ls + 1) * 128],
                        c2_sb[:, g * H + nh * 512: g * H + nh * 512 + 512],
                        start=(g == 0), stop=(g == 3))
            for nh in range(2):
                stt_v(ob[:, nh * 512:(nh + 1) * 512], pcs[nh][:], 1.0,
                      xd[:, nh * 512:(nh + 1) * 512], MULT, ADD)
            nc.sync.dma_start(out_d[lt * 128:(lt + 1) * 128, :], ob[:])

    nc.finalize()
    return nc


def _get_program():
    import os
    if "nc" not in _prog_cache:
        g = os.environ.get("K_GPSIMD_CTL", "0") == "1"
        _prog_cache["nc"] = _build_program(gpsimd_ctl=g)
    return _prog_cache["nc"]


def make_in_maps(input_sequence, steps_raw, G_raw, A_raw, B_mat, C_mat, D_vec):
    a, b, c0, c1 = _host_params(steps_raw, G_raw, A_raw)
    par = _pack_params(a, b, c0, c1)
    Bm = B_mat.astype(np.float64)   # (P, H, 2)
    Cm = C_mat.astype(np.float64)   # (H, P, 2)
    Bw = np.concatenate([Bm[:, :, 0].T, Bm[:, :, 1].T], axis=1)  # (H, 2P)
    C2 = np.concatenate([Cm[:, :, 0].T, -Cm[:, :, 1].T], axis=0)  # (2P, H)
    bw16 = np.ascontiguousarray(Bw.astype(ml_dtypes.bfloat16))
    c216 = np.ascontiguousarray(C2.astype(ml_dtypes.bfloat16))
    drep = np.broadcast_to(
        D_vec.astype(np.float32).astype(ml_dtypes.bfloat16), (128, H)).copy()
    xb16 = np.ascontiguousarray(
        np.asarray(input_sequence, np.float32).astype(ml_dtypes.bfloat16))
    in_maps = []
    for i in range(B):
        in_maps.append({
            "xb16": xb16[i],
            "bw": bw16,
            "c2": c216,
            "drep": drep,
            "par": par,
        })
    return in_maps


def kernel(input_sequence, steps_raw, G_raw, A_raw, B_mat, C_mat, D_vec):
    from concourse.bass_utils import run_bass_kernel_spmd

    nc = _get_program()
    in_maps = make_in_maps(input_sequence, steps_raw, G_raw, A_raw,
                           B_mat, C_mat, D_vec)
    res = run_bass_kernel_spmd(nc, in_maps, core_ids=list(range(B)))
    out = np.stack([res.results[i]["out"] for i in range(B)], axis=0)
    return out.astype(np.float32)
